# revision 1
# baseline (speedup 1.0000x reference)
"""DCGRU cell (DCRNN) Trainium2 Bass kernel.

Strategy (see spec sharding_hint): data-parallel over batch B=64 across 8
NeuronCores (8 batches per core); supports + gconv weights replicated.

Math restructuring (validated in numpy against the jax reference):
  reference diffusion xs = [x0, S0@x0, 2*S0^2@x0 - x0, S1@S0@x0, 2*S1^2@S0@x0 - S0@x0]
  -> raw chain     ys = [y0, y1=S0@y0, y2=S0@y1, y3=S1@y1, y4=S1@y3]
  with the 2a-b combinations folded into the projection weights on the host:
  What = [W0-W2, W1-W4, 2*W2, W3, 2*W4] (Wm = rows insz*5+m of the gconv W).

Per-core device layout:
  Diffusion state X [N, 528] in SBUF, columns c = b*64+u (hx part, b=0..7)
  then 512 + b*2 + j (input part).  Hops are PE matmuls out[nb-block, c] +=
  ST_tile[kb,nb].T @ X[kb-block, c] with host-pretransposed, block-packed
  supports streamed from HBM (the memory roofline of this problem).
  After each hop the result is transposed on PE (128x128 chunks) and spilled
  to DRAM as YT [528, N] so the projection can contract over features with
  the feature dim on partitions.  Projection: ZT_b[out,n] = sum_m
  What_m.T @ YT_m[b-rows, n] accumulated in PSUM, fused bias+sigmoid/tanh on
  ACT, gate arithmetic on DVE, all in [units, n] layout; host un-transposes
  the final output during unsharding.
Matmuls run as float32r (full PE rate, fp32 storage).
"""

import os
from contextlib import ExitStack

import numpy as np

import concourse.bacc as bacc
import concourse.mybir as mybir
import concourse.tile as tile
from concourse.bass_utils import run_bass_kernel_spmd
from concourse.masks import make_identity

F32 = mybir.dt.float32
F32R = mybir.dt.float32r


def _r(ap):
    return ap.bitcast(F32R)

NCORES = 8
B = 64
BLOC = B // NCORES  # 8
IN_DIM = 2
UNITS = 64
CHX = BLOC * UNITS  # 512
C = CHX + BLOC * IN_DIM  # 528
CIN = BLOC * IN_DIM  # 16
CH = C // 2  # 264 (psum free-dim split)


def _build_nc(N):
    """Build the per-core Bass program (SPMD; same NEFF on all 8 cores)."""
    NB = N // 128  # row blocks (32 at full size)
    PCH = min(2048, N)  # phase-P n-chunk held in SBUF
    NHALF = N // PCH
    NFC = PCH // 512  # 512-wide proj chunks per PCH

    nc = bacc.Bacc("TRN2", target_bir_lowering=False, debug=False)

    # ---- external I/O ----
    x0pm = nc.dram_tensor("x0pm", [128, NB * C], F32, kind="ExternalInput").ap()
    stb = nc.dram_tensor("stb", [2, NB, 128, NB * 128], F32, kind="ExternalInput").ap()
    xint = nc.dram_tensor("xint", [CIN, N], F32, kind="ExternalInput").ap()
    hxt = nc.dram_tensor("hxt", [BLOC, UNITS, N], F32, kind="ExternalInput").ap()
    wfn = nc.dram_tensor("wfn", [66, 5 * 128], F32, kind="ExternalInput").ap()
    wg = nc.dram_tensor("wg", [66, 5 * 64], F32, kind="ExternalInput").ap()
    bfn = nc.dram_tensor("bfn", [128, 1], F32, kind="ExternalInput").ap()
    bg = nc.dram_tensor("bg", [64, 1], F32, kind="ExternalInput").ap()
    outt = nc.dram_tensor("outt", [BLOC, UNITS, N], F32, kind="ExternalOutput").ap()

    with tile.TileContext(nc) as tc, ExitStack() as ctx:
        # ---- persistent pools ----
        const = ctx.enter_context(tc.tile_pool(name="const", bufs=1))
        dram = ctx.enter_context(tc.tile_pool(name="dram", bufs=1, space="DRAM"))

        ident = const.tile([128, 128], F32, name="ident")
        make_identity(nc, ident)
        wfn_sb = const.tile([66, 5 * 128], F32, name="wfn_sb")
        nc.sync.dma_start(_r(wfn_sb), _r(wfn))
        wg_sb = const.tile([66, 5 * 64], F32, name="wg_sb")
        nc.sync.dma_start(_r(wg_sb), _r(wg))
        bfn_sb = const.tile([128, 1], F32, name="bfn_sb")
        nc.sync.dma_start(bfn_sb, bfn)
        bg_sb = const.tile([64, 1], F32, name="bg_sb")
        nc.sync.dma_start(bg_sb, bg)
        # DRAM scratch: transposed diffusion results per gconv/hop, u gate,
        # rebuilt x0 for gconv2.
        # 640 = 5*128 rows: rows 0:512 hx-part, 512:528 input-part, rest pad
        # (padding lets each block spill as ONE 5x128x128 DMA).
        ytd = [
            [
                dram.tile([640, N], F32, name=f"ytd_{g}_{m}", tag=f"ytd_{g}_{m}")
                for m in range(1, 5)
            ]
            for g in range(2)
        ]
        yt0p = dram.tile([CHX, N], F32, name="yt0p", tag="yt0p")
        x0p = dram.tile([128, BLOC * NB * UNITS], F32, name="x0p", tag="x0p")
        u_d = dram.tile([BLOC, UNITS, N], F32, name="u_d", tag="u_d")

        def diffusion(g):
            """4 hops; X0 loaded from DRAM (x0pm for g=0, x0p for g=1)."""
            with (
                tc.tile_pool(name=f"ybuf{g}", bufs=1) as yp,
                tc.tile_pool(name=f"st{g}", bufs=2) as stp,
                tc.tile_pool(name=f"dps{g}", bufs=2, space="PSUM") as dps,
                tc.tile_pool(name=f"tps{g}", bufs=2, space="PSUM") as tps,
                tc.tile_pool(name=f"yts{g}", bufs=3) as ytsp,
            ):
                bufA = yp.tile([128, NB * C], F32, name=f"bufA{g}", tag="bufA")
                bufB = yp.tile([128, NB * C], F32, name=f"bufB{g}", tag="bufB")
                if g == 0:
                    q4 = NB * C // 4
                    for q in range(4):
                        nc.sync.dma_start(
                            _r(bufA[:, q * q4 : (q + 1) * q4]),
                            _r(x0pm[:, q * q4 : (q + 1) * q4]),
                        )
                else:
                    # x0p is stored b-major [b, kb, u]; diffusion layout is
                    # [kb, b*64+u] with stride C -- one DMA per b
                    for b in range(BLOC):
                        nc.sync.dma_start(
                            _r(
                                bufA.rearrange("p (k c) -> p k c", c=C)[
                                    :, :, b * UNITS : (b + 1) * UNITS
                                ]
                            ),
                            _r(
                                x0p[
                                    :, b * NB * UNITS : (b + 1) * NB * UNITS
                                ].rearrange("p (k u) -> p k u", u=UNITS)
                            ),
                        )

                # gconv2 skips the 16 input columns entirely: their diffusion
                # is identical to gconv1's, so phase P reuses g1's spills.
                W = C if g == 0 else CHX
                HW_ = W // 2  # 264 (g1) / 256 (g2) psum free split
                NJ = 5 if g == 0 else 4  # spill row-chunks

                def hop(src, dst, s_idx, yt_dst):
                    def compute_block(nb):
                        slab = stp.tile(
                            [128, NB * 128], F32, name=f"slab{g}", tag="slab"
                        )
                        nc.sync.dma_start(_r(slab), _r(stb[s_idx, nb]))
                        if g == 0:
                            # 528 cols: two 264-wide psum groups (>512 limit)
                            pa = dps.tile([128, HW_], F32, name=f"pa{g}", tag="pa")
                            pb = dps.tile([128, HW_], F32, name=f"pb{g}", tag="pb")
                            for kb in range(NB):
                                lh = slab[:, kb * 128 : (kb + 1) * 128].bitcast(F32R)
                                nc.tensor.matmul(
                                    pa,
                                    lh,
                                    src[:, kb * C : kb * C + HW_].bitcast(F32R),
                                    start=(kb == 0),
                                    stop=(kb == NB - 1),
                                )
                                nc.tensor.matmul(
                                    pb,
                                    lh,
                                    src[:, kb * C + HW_ : kb * C + W].bitcast(F32R),
                                    start=(kb == 0),
                                    stop=(kb == NB - 1),
                                )
                            nc.vector.tensor_copy(
                                _r(dst[:, nb * C : nb * C + HW_]), pa
                            )
                            nc.vector.tensor_copy(
                                _r(dst[:, nb * C + HW_ : nb * C + W]), pb
                            )
                        else:
                            # 512 cols fit one psum bank: single matmul per
                            # tile -> half the weight loads
                            pa = dps.tile([128, W], F32, name=f"pa{g}", tag="pa")
                            for kb in range(NB):
                                nc.tensor.matmul(
                                    pa,
                                    slab[:, kb * 128 : (kb + 1) * 128].bitcast(F32R),
                                    src[:, kb * C : kb * C + W].bitcast(F32R),
                                    start=(kb == 0),
                                    stop=(kb == NB - 1),
                                )
                            nc.vector.tensor_copy(_r(dst[:, nb * C : nb * C + W]), pa)

                    def transpose_block(nb):
                        # transpose the block's columns into one staging
                        # tile, spill with a single chunked DMA
                        yts = ytsp.tile(
                            [128, NJ * 128], F32, name=f"yts{g}", tag="yts"
                        )
                        for j in range(4):
                            tpp = tps.tile([128, 128], F32, name=f"tpp{g}", tag="tpp")
                            nc.tensor.transpose(
                                tpp,
                                dst[:, nb * C + j * 128 : nb * C + (j + 1) * 128],
                                ident,
                            )
                            nc.vector.tensor_copy(
                                _r(yts[:, j * 128 : (j + 1) * 128]), tpp
                            )
                        if g == 0:
                            tpi = tps.tile([128, 128], F32, name=f"tpi{g}", tag="tpp")
                            nc.tensor.transpose(
                                tpi[:CIN, :],
                                dst[:, nb * C + CHX : (nb + 1) * C],
                                ident,
                            )
                            nc.vector.tensor_copy(
                                _r(yts[:CIN, 512:640]), tpi[:CIN, :]
                            )
                        nc.scalar.dma_start(
                            _r(
                                yt_dst[
                                    : NJ * 128, nb * 128 : (nb + 1) * 128
                                ].rearrange("(j r) n -> r j n", r=128)
                            ),
                            _r(yts.rearrange("p (j c) -> p j c", c=128)),
                        )

                    # transposes deferred by 2 blocks so PE never stalls on
                    # the DVE psum-copies feeding them
                    for nb in range(NB):
                        compute_block(nb)
                        if nb >= 2:
                            transpose_block(nb - 2)
                    transpose_block(NB - 2)
                    transpose_block(NB - 1)

                hop(bufA, bufB, 0, ytd[g][0])  # y1 = S0 @ y0
                hop(bufB, bufA, 0, ytd[g][1])  # y2 = S0 @ y1
                hop(bufB, bufA, 1, ytd[g][2])  # y3 = S1 @ y1 (y2 spilled)
                hop(bufA, bufB, 1, ytd[g][3])  # y4 = S1 @ y3

        def projection(g):
            D = 128 if g == 0 else 64
            w_sb = wfn_sb if g == 0 else wg_sb
            with (
                tc.tile_pool(name=f"ytp{g}", bufs=12) as ytp,
                tc.tile_pool(name=f"aux{g}", bufs=4) as aux,
                tc.tile_pool(name=f"zps{g}", bufs=4, space="PSUM") as zps,
                tc.tile_pool(name=f"tpq{g}", bufs=3, space="PSUM") as tpq,
            ):
                for b in range(BLOC):
                    for half in range(NHALF):
                        ns = half * PCH
                        if g == 1:
                            hx_t = aux.tile(
                                [UNITS, PCH], F32, name=f"hx_t{g}", tag="hx_t", bufs=3
                            )
                            nc.sync.dma_start(hx_t, hxt[b, :, ns : ns + PCH])
                            u_t = aux.tile([UNITS, PCH], F32, name="u_t", tag="u_t", bufs=3)
                            nc.gpsimd.dma_start(u_t, u_d[b, :, ns : ns + PCH])
                        yts = []
                        for m in range(5):
                            yt_t = ytp.tile([66, PCH], F32, name=f"yt{g}", tag="yt")
                            if m == 0:
                                hx_src = (
                                    hxt[b, :, ns : ns + PCH]
                                    if g == 0
                                    else yt0p[b * UNITS : (b + 1) * UNITS, ns : ns + PCH]
                                )
                                in_src = xint[b * 2 : b * 2 + 2, ns : ns + PCH]
                            else:
                                ytm = ytd[g][m - 1]
                                hx_src = ytm[b * UNITS : (b + 1) * UNITS, ns : ns + PCH]
                                in_src = ytd[0][m - 1][
                                    CHX + b * 2 : CHX + b * 2 + 2, ns : ns + PCH
                                ]
                            eng = nc.sync if m % 2 == 0 else nc.scalar
                            eng.dma_start(_r(yt_t[0:UNITS, :]), _r(hx_src))
                            eng.dma_start(_r(yt_t[UNITS:66, :]), _r(in_src))
                            yts.append(yt_t)
                        for nfc in range(NFC):
                            zp = zps.tile([D, 512], F32, name=f"zp{g}", tag="zp")
                            for m in range(5):
                                nc.tensor.matmul(
                                    zp,
                                    w_sb[:, m * D : (m + 1) * D].bitcast(F32R),
                                    yts[m][:, nfc * 512 : (nfc + 1) * 512].bitcast(
                                        F32R
                                    ),
                                    start=(m == 0),
                                    stop=(m == 4),
                                )
                            nf0 = ns + nfc * 512
                            if g == 0:
                                val = aux.tile([128, 512], F32, name="val", tag="val")
                                nc.scalar.activation(
                                    val,
                                    zp,
                                    mybir.ActivationFunctionType.Sigmoid,
                                    bias=bfn_sb,
                                )
                                rh = aux.tile([64, 512], F32, name="rh", tag="rh")
                                nc.vector.tensor_mul(
                                    _r(rh),
                                    val[0:64, :],
                                    yts[0][0:UNITS, nfc * 512 : (nfc + 1) * 512],
                                )
                                nc.gpsimd.dma_start(
                                    u_d[b, :, nf0 : nf0 + 512], val[64:128, :]
                                )
                                nc.gpsimd.dma_start(
                                    _r(
                                        yt0p[
                                            b * UNITS : (b + 1) * UNITS, nf0 : nf0 + 512
                                        ]
                                    ),
                                    _r(rh),
                                )
                                # un-transpose r*hx into gconv2's diffusion layout
                                xs4 = aux.tile([128, 4, 64], F32, name="xs4", tag="xs4")
                                for sub in range(4):
                                    tpp = tpq.tile(
                                        [128, 128], F32, name="tpq_t", tag="tpq"
                                    )
                                    nc.tensor.transpose(
                                        tpp[:, 0:64],
                                        rh[:, sub * 128 : (sub + 1) * 128],
                                        ident[0:64, 0:64],
                                    )
                                    nc.vector.tensor_copy(_r(xs4[:, sub, :]), tpp[:, 0:64])
                                kb0 = nf0 // 128
                                o0 = (b * NB + kb0) * UNITS
                                nc.gpsimd.dma_start(
                                    _r(x0p[:, o0 : o0 + 4 * UNITS]),
                                    _r(xs4.rearrange("p s u -> p (s u)")),
                                )
                            else:
                                ct = aux.tile([64, 512], F32, name="ct", tag="ct")
                                nc.scalar.activation(
                                    ct, zp, mybir.ActivationFunctionType.Tanh, bias=bg_sb
                                )
                                tmp = aux.tile([64, 512], F32, name="tmp", tag="tmp")
                                nc.vector.tensor_sub(
                                    tmp, hx_t[:, nfc * 512 : (nfc + 1) * 512], ct
                                )
                                nc.vector.tensor_mul(
                                    tmp, tmp, u_t[:, nfc * 512 : (nfc + 1) * 512]
                                )
                                ot = aux.tile([64, 512], F32, name="ot", tag="ot")
                                nc.vector.tensor_add(ot, tmp, ct)
                                nc.gpsimd.dma_start(outt[b, :, nf0 : nf0 + 512], ot)

        diffusion(0)
        projection(0)
        diffusion(1)
        projection(1)

    nc.compile()
    return nc


def _fold_weights(w, out_dim):
    """w: (330, out). Returns [66, 5*out] with the reference's x0c-mutation
    linear combinations folded in and rows reordered hx-first."""
    Wm = w.reshape(66, 5, out_dim)
    What = np.stack(
        [
            Wm[:, 0] - Wm[:, 2],
            Wm[:, 1] - Wm[:, 4],
            2.0 * Wm[:, 2],
            Wm[:, 3],
            2.0 * Wm[:, 4],
        ]
    )  # [5, 66, out]
    What = np.concatenate([What[:, 2:, :], What[:, :2, :]], axis=1)  # hx rows first
    return np.ascontiguousarray(
        What.transpose(1, 0, 2).reshape(66, 5 * out_dim)
    ).astype(np.float32)


_NC_CACHE = {}


def _get_nc(N):
    if N not in _NC_CACHE:
        _NC_CACHE[N] = _build_nc(N)
    return _NC_CACHE[N]


def kernel(inputs, hx, supports, w_fn, b_fn, w_g, b_g):
    inputs = np.ascontiguousarray(np.asarray(inputs), dtype=np.float32)
    hx = np.ascontiguousarray(np.asarray(hx), dtype=np.float32)
    supports = np.ascontiguousarray(np.asarray(supports), dtype=np.float32)
    w_fn = np.asarray(w_fn, dtype=np.float32)
    b_fn = np.asarray(b_fn, dtype=np.float32)
    w_g = np.asarray(w_g, dtype=np.float32)
    b_g = np.asarray(b_g, dtype=np.float32)

    N = supports.shape[1]
    NB = N // 128
    nc = _get_nc(N)

    # ---- replicated tensors ----
    # stb[s, nb, kp, kb*128+m] = supports[s][nb*128+m, kb*128+kp]
    stb = np.ascontiguousarray(
        supports.reshape(2, NB, 128, NB, 128).transpose(0, 1, 4, 3, 2)
    ).reshape(2, NB, 128, NB * 128)
    wfn_h = _fold_weights(w_fn, 128)
    wg_h = _fold_weights(w_g, 64)
    bfn_h = b_fn.reshape(128, 1).copy()
    bg_h = b_g.reshape(64, 1).copy()

    in_maps = []
    for c in range(NCORES):
        sl = slice(c * BLOC, (c + 1) * BLOC)
        inp_c = inputs[sl].reshape(BLOC, N, IN_DIM)
        hx_c = hx[sl].reshape(BLOC, N, UNITS)
        # X0 [N, 528]: hx cols b*64+u, input cols 512 + b*2 + j
        x0 = np.concatenate(
            [
                hx_c.transpose(1, 0, 2).reshape(N, CHX),
                inp_c.transpose(1, 0, 2).reshape(N, CIN),
            ],
            axis=1,
        )
        x0pm = np.ascontiguousarray(
            x0.reshape(NB, 128, C).transpose(1, 0, 2)
        ).reshape(128, NB * C)
        xin = x0[:, CHX:]
        xint = np.ascontiguousarray(xin.T)
        hxt = np.ascontiguousarray(hx_c.transpose(0, 2, 1))
        in_maps.append(
            {
                "x0pm": x0pm,
                "stb": stb,
                "xint": xint,
                "hxt": hxt,
                "wfn": wfn_h,
                "wg": wg_h,
                "bfn": bfn_h,
                "bg": bg_h,
            }
        )

    kernel.last_in_maps = in_maps
    res = run_bass_kernel_spmd(
        nc,
        in_maps,
        core_ids=list(range(NCORES)),
        trace=bool(int(os.environ.get("DCGRU_TRACE", "0"))),
    )

    out = np.empty((B, N * UNITS), np.float32)
    for c in range(NCORES):
        outt = res.results[c]["outt"]  # [BLOC, UNITS, N]
        out[c * BLOC : (c + 1) * BLOC] = outt.transpose(0, 2, 1).reshape(BLOC, -1)
    kernel.last_results = res
    return out



# revision 3
# speedup vs baseline: 6.9680x; 6.9680x over previous
"""DCGRU cell (DCRNN) Trainium2 Bass kernel.

Strategy (see spec sharding_hint): data-parallel over batch B=64 across 8
NeuronCores (8 batches per core); supports + gconv weights replicated.

Math restructuring (validated in numpy against the jax reference):
  reference diffusion xs = [x0, S0@x0, 2*S0^2@x0 - x0, S1@S0@x0, 2*S1^2@S0@x0 - S0@x0]
  -> raw chain     ys = [y0, y1=S0@y0, y2=S0@y1, y3=S1@y1, y4=S1@y3]
  with the 2a-b combinations folded into the projection weights on the host:
  What = [W0-W2, W1-W4, 2*W2, W3, 2*W4] (Wm = rows insz*5+m of the gconv W).

Quantization (validated in numpy: rel err ~2e-4 vs fp32 reference):
  The diffusion chain runs in fp8e4 with MatmulPerfMode.DoubleRow (2
  k-subtiles per matmul, 2x PE throughput vs fp32r/bf16).  S entries are
  ~2.4e-4 (below fp8 subnormal range) so supports are scaled by 2^11 on the
  host; hop outputs are descaled and restored to fp8 with value scale 2^5
  (diffused stds ~0.015) by a fused scaled-copy on the ACT engine.  The 2^-5
  storage scale is folded into the projection weights per diffusion matrix.
  The projection runs in bf16 (weights, YT spills, m=0 operands); the final
  output terms are diluted ~50x relative to the diffusion values, so the
  fp8 chain error contributes only ~2e-4 to the final relative error.

Per-core device layout:
  Diffusion state X [N, 528] fp8 in SBUF, columns c = b*64+u (hx part,
  b=0..7) then 512 + b*2 + j (input part).  Hops are PE DoubleRow matmuls
  out[nb-block, c] += ST_tile[2 k-subtiles].T @ X[2 k-subtiles, c] with
  host-pretransposed, block-packed fp8 supports streamed from HBM.
  After each hop the result is transposed on PE (fp8, 128x128 chunks) and
  spilled to DRAM as YT [528-ish, N] bf16 so the projection can contract
  over features with the feature dim on partitions.  Projection:
  ZT_b[out,n] = sum_m What_m.T @ YT_m[b-rows, n] accumulated in PSUM (bf16
  operands), fused bias+sigmoid/tanh on ACT, gate arithmetic on DVE in
  fp32, all in [units, n] layout; host un-transposes the final output
  during unsharding.
"""

import os
from contextlib import ExitStack

import numpy as np
import ml_dtypes

import concourse.bacc as bacc
import concourse.mybir as mybir
import concourse.tile as tile
from concourse.bass_utils import run_bass_kernel_spmd
from concourse.masks import make_identity

F32 = mybir.dt.float32
F32R = mybir.dt.float32r
BF16 = mybir.dt.bfloat16
F8 = mybir.dt.float8e4
DR = mybir.MatmulPerfMode.DoubleRow

NP_F8 = ml_dtypes.float8_e4m3
NP_BF16 = ml_dtypes.bfloat16

S_SCALE = 2.0**11  # host: supports scaled into fp8 normal range
Y_SCALE = 2.0**5  # stored scale of diffused chain values (stds ~0.015)
# ACT descale on the psum->fp8 hop copy: hop 0 input is unscaled (y0),
# hops >=1 input carries Y_SCALE.
COPY_SCALE_H0 = Y_SCALE / S_SCALE
COPY_SCALE = Y_SCALE / (S_SCALE * Y_SCALE)


def _r(ap):
    return ap.bitcast(F32R)

NCORES = 8
B = 64
BLOC = B // NCORES  # 8
IN_DIM = 2
UNITS = 64
CHX = BLOC * UNITS  # 512
C = CHX + BLOC * IN_DIM  # 528
CIN = BLOC * IN_DIM  # 16
CH = C // 2  # 264 (psum free-dim split)


def _build_nc(N):
    """Build the per-core Bass program (SPMD; same NEFF on all 8 cores)."""
    NB = N // 128  # row blocks (32 at full size)
    PCH = min(2048, N)  # phase-P n-chunk held in SBUF
    NHALF = N // PCH
    NFC = PCH // 512  # 512-wide proj chunks per PCH

    nc = bacc.Bacc("TRN2", target_bir_lowering=False, debug=False)

    # ---- external I/O ----
    x0pm = nc.dram_tensor("x0pm", [128, NB * C], F8, kind="ExternalInput").ap()
    stb = nc.dram_tensor("stb", [2, NB, 128, NB * 128], F8, kind="ExternalInput").ap()
    xint = nc.dram_tensor("xint", [CIN, N], BF16, kind="ExternalInput").ap()
    hxt = nc.dram_tensor("hxt", [BLOC, UNITS, N], F32, kind="ExternalInput").ap()
    hxtb = nc.dram_tensor("hxtb", [BLOC, UNITS, N], BF16, kind="ExternalInput").ap()
    wfn = nc.dram_tensor("wfn", [66, 5 * 128], BF16, kind="ExternalInput").ap()
    wg = nc.dram_tensor("wg", [66, 5 * 64], BF16, kind="ExternalInput").ap()
    bfn = nc.dram_tensor("bfn", [128, 1], F32, kind="ExternalInput").ap()
    bg = nc.dram_tensor("bg", [64, 1], F32, kind="ExternalInput").ap()
    outt = nc.dram_tensor("outt", [BLOC, UNITS, N], F32, kind="ExternalOutput").ap()

    with tile.TileContext(nc) as tc, ExitStack() as ctx:
        # ---- persistent pools ----
        const = ctx.enter_context(tc.tile_pool(name="const", bufs=1))
        dram = ctx.enter_context(tc.tile_pool(name="dram", bufs=1, space="DRAM"))

        ident8 = const.tile([128, 128], F8, name="ident8")
        make_identity(nc, ident8)
        identb = const.tile([128, 128], BF16, name="identb")
        make_identity(nc, identb)
        wfn_sb = const.tile([66, 5 * 128], BF16, name="wfn_sb")
        nc.sync.dma_start(wfn_sb, wfn)
        wg_sb = const.tile([66, 5 * 64], BF16, name="wg_sb")
        nc.sync.dma_start(wg_sb, wg)
        bfn_sb = const.tile([128, 1], F32, name="bfn_sb")
        nc.sync.dma_start(bfn_sb, bfn)
        bg_sb = const.tile([64, 1], F32, name="bg_sb")
        nc.sync.dma_start(bg_sb, bg)
        # DRAM scratch: transposed diffusion results per gconv/hop (bf16), u
        # gate, rebuilt x0 (fp8) for gconv2.
        # 640 = 5*128 rows: rows 0:512 hx-part, 512:528 input-part, rest pad
        # (padding lets each block spill as ONE 5x128x128 DMA).
        ytd = [
            [
                dram.tile([640, N], BF16, name=f"ytd_{g}_{m}", tag=f"ytd_{g}_{m}")
                for m in range(1, 5)
            ]
            for g in range(2)
        ]
        yt0p = dram.tile([CHX, N], BF16, name="yt0p", tag="yt0p")
        x0p = dram.tile([128, BLOC * NB * UNITS], F8, name="x0p", tag="x0p")
        u_d = dram.tile([BLOC, UNITS, N], F32, name="u_d", tag="u_d")

        def diffusion(g):
            """4 hops; X0 loaded from DRAM (x0pm for g=0, x0p for g=1)."""
            with (
                tc.tile_pool(name=f"ybuf{g}", bufs=1) as yp,
                tc.tile_pool(name=f"st{g}", bufs=2) as stp,
                tc.tile_pool(name=f"dps{g}", bufs=2, space="PSUM") as dps,
                tc.tile_pool(name=f"tps{g}", bufs=2, space="PSUM") as tps,
                tc.tile_pool(name=f"yts{g}", bufs=3) as ytsp,
            ):
                bufA = yp.tile([128, NB * C], F8, name=f"bufA{g}", tag="bufA")
                bufB = yp.tile([128, NB * C], F8, name=f"bufB{g}", tag="bufB")
                if g == 0:
                    q4 = NB * C // 4
                    for q in range(4):
                        nc.sync.dma_start(
                            bufA[:, q * q4 : (q + 1) * q4],
                            x0pm[:, q * q4 : (q + 1) * q4],
                        )
                else:
                    # x0p is stored b-major [b, kb, u]; diffusion layout is
                    # [kb, b*64+u] with stride C -- one DMA per b
                    for b in range(BLOC):
                        nc.sync.dma_start(
                            bufA.rearrange("p (k c) -> p k c", c=C)[
                                :, :, b * UNITS : (b + 1) * UNITS
                            ],
                            x0p[
                                :, b * NB * UNITS : (b + 1) * NB * UNITS
                            ].rearrange("p (k u) -> p k u", u=UNITS),
                        )

                # gconv2 skips the 16 input columns entirely: their diffusion
                # is identical to gconv1's, so phase P reuses g1's spills.
                W = C if g == 0 else CHX
                HW_ = W // 2  # 264 (g1) / 256 (g2) psum free split
                NJ = 5 if g == 0 else 4  # spill row-chunks

                def hop(src, dst, s_idx, yt_dst, cscale):
                    src3 = src.rearrange("p (k c) -> p k c", c=C)

                    def compute_block(nb):
                        slab = stp.tile(
                            [128, NB * 128], F8, name=f"slab{g}", tag="slab"
                        )
                        nc.sync.dma_start(slab, stb[s_idx, nb])
                        slab3 = slab.rearrange("p (k m) -> p k m", m=128)
                        # DoubleRow fp8: 2 k-subtiles per matmul, 0.5 cyc/row
                        pa = dps.tile([128, HW_], F32, name=f"pa{g}", tag="pa")
                        pb = dps.tile([128, HW_], F32, name=f"pb{g}", tag="pb")
                        for ki in range(NB // 2):
                            lh = slab3[:, 2 * ki : 2 * ki + 2, :]
                            nc.tensor.matmul(
                                pa,
                                lh,
                                src3[:, 2 * ki : 2 * ki + 2, 0:HW_],
                                start=(ki == 0),
                                stop=(ki == NB // 2 - 1),
                                perf_mode=DR,
                            )
                            nc.tensor.matmul(
                                pb,
                                lh,
                                src3[:, 2 * ki : 2 * ki + 2, HW_:W],
                                start=(ki == 0),
                                stop=(ki == NB // 2 - 1),
                                perf_mode=DR,
                            )
                        # fused descale + fp32->fp8 store on ACT
                        nc.scalar.activation(
                            dst[:, nb * C : nb * C + HW_],
                            pa,
                            mybir.ActivationFunctionType.Copy,
                            scale=cscale,
                        )
                        nc.scalar.activation(
                            dst[:, nb * C + HW_ : nb * C + W],
                            pb,
                            mybir.ActivationFunctionType.Copy,
                            scale=cscale,
                        )

                    def transpose_block(nb):
                        # transpose the block's columns (fp8) into one staging
                        # tile (bf16), spill with a single chunked DMA
                        yts = ytsp.tile(
                            [128, NJ * 128], BF16, name=f"yts{g}", tag="yts"
                        )
                        for j in range(4):
                            # fp8 transpose writes one value per 2-byte lane:
                            # output AP must have element step 2
                            tpp = tps.tile([128, 256], F8, name=f"tpp{g}", tag="tpp")
                            tppv = tpp.rearrange("p (c t) -> p c t", t=2)[:, :, 0:1]
                            nc.tensor.transpose(
                                tppv,
                                dst[:, nb * C + j * 128 : nb * C + (j + 1) * 128],
                                ident8,
                            )
                            nc.vector.tensor_copy(
                                yts[:, j * 128 : (j + 1) * 128], tppv
                            )
                        if g == 0:
                            tpi = tps.tile([128, 256], F8, name=f"tpi{g}", tag="tpp")
                            tpiv = tpi.rearrange("p (c t) -> p c t", t=2)[
                                :CIN, :, 0:1
                            ]
                            nc.tensor.transpose(
                                tpiv,
                                dst[:, nb * C + CHX : (nb + 1) * C],
                                ident8,
                            )
                            nc.vector.tensor_copy(yts[:CIN, 512:640], tpiv)
                        nc.scalar.dma_start(
                            yt_dst[
                                : NJ * 128, nb * 128 : (nb + 1) * 128
                            ].rearrange("(j r) n -> r j n", r=128),
                            yts.rearrange("p (j c) -> p j c", c=128),
                        )

                    # transposes deferred by 2 blocks so PE never stalls on
                    # the DVE psum-copies feeding them
                    for nb in range(NB):
                        compute_block(nb)
                        if nb >= 2:
                            transpose_block(nb - 2)
                    transpose_block(NB - 2)
                    transpose_block(NB - 1)

                hop(bufA, bufB, 0, ytd[g][0], COPY_SCALE_H0)  # y1 = S0 @ y0
                hop(bufB, bufA, 0, ytd[g][1], COPY_SCALE)  # y2 = S0 @ y1
                hop(bufB, bufA, 1, ytd[g][2], COPY_SCALE)  # y3 = S1 @ y1
                hop(bufA, bufB, 1, ytd[g][3], COPY_SCALE)  # y4 = S1 @ y3

        def projection(g):
            D = 128 if g == 0 else 64
            w_sb = wfn_sb if g == 0 else wg_sb
            with (
                tc.tile_pool(name=f"ytp{g}", bufs=12) as ytp,
                tc.tile_pool(name=f"aux{g}", bufs=4) as aux,
                tc.tile_pool(name=f"zps{g}", bufs=4, space="PSUM") as zps,
                tc.tile_pool(name=f"tpq{g}", bufs=3, space="PSUM") as tpq,
            ):
                for b in range(BLOC):
                    for half in range(NHALF):
                        ns = half * PCH
                        hx_t = aux.tile(
                            [UNITS, PCH], F32, name=f"hx_t{g}", tag="hx_t", bufs=3
                        )
                        nc.sync.dma_start(hx_t, hxt[b, :, ns : ns + PCH])
                        if g == 1:
                            u_t = aux.tile([UNITS, PCH], F32, name="u_t", tag="u_t", bufs=3)
                            nc.gpsimd.dma_start(u_t, u_d[b, :, ns : ns + PCH])
                        yts = []
                        for m in range(5):
                            yt_t = ytp.tile([66, PCH], BF16, name=f"yt{g}", tag="yt")
                            if m == 0:
                                hx_src = (
                                    hxtb[b, :, ns : ns + PCH]
                                    if g == 0
                                    else yt0p[b * UNITS : (b + 1) * UNITS, ns : ns + PCH]
                                )
                                in_src = xint[b * 2 : b * 2 + 2, ns : ns + PCH]
                            else:
                                ytm = ytd[g][m - 1]
                                hx_src = ytm[b * UNITS : (b + 1) * UNITS, ns : ns + PCH]
                                in_src = ytd[0][m - 1][
                                    CHX + b * 2 : CHX + b * 2 + 2, ns : ns + PCH
                                ]
                            eng = nc.sync if m % 2 == 0 else nc.scalar
                            eng.dma_start(yt_t[0:UNITS, :], hx_src)
                            eng.dma_start(yt_t[UNITS:66, :], in_src)
                            yts.append(yt_t)
                        for nfc in range(NFC):
                            zp = zps.tile([D, 512], F32, name=f"zp{g}", tag="zp")
                            for m in range(5):
                                nc.tensor.matmul(
                                    zp,
                                    w_sb[:, m * D : (m + 1) * D],
                                    yts[m][:, nfc * 512 : (nfc + 1) * 512],
                                    start=(m == 0),
                                    stop=(m == 4),
                                )
                            nf0 = ns + nfc * 512
                            if g == 0:
                                val = aux.tile([128, 512], F32, name="val", tag="val")
                                nc.scalar.activation(
                                    val,
                                    zp,
                                    mybir.ActivationFunctionType.Sigmoid,
                                    bias=bfn_sb,
                                )
                                rh = aux.tile([64, 512], BF16, name="rh", tag="rh")
                                nc.vector.tensor_mul(
                                    rh,
                                    val[0:64, :],
                                    hx_t[:, nfc * 512 : (nfc + 1) * 512],
                                )
                                nc.gpsimd.dma_start(
                                    u_d[b, :, nf0 : nf0 + 512], val[64:128, :]
                                )
                                nc.gpsimd.dma_start(
                                    yt0p[
                                        b * UNITS : (b + 1) * UNITS, nf0 : nf0 + 512
                                    ],
                                    rh,
                                )
                                # un-transpose r*hx into gconv2's diffusion
                                # layout (fp8)
                                xs4 = aux.tile([128, 4, 64], F8, name="xs4", tag="xs4")
                                for sub in range(4):
                                    tpp = tpq.tile(
                                        [128, 128], BF16, name="tpq_t", tag="tpq"
                                    )
                                    nc.tensor.transpose(
                                        tpp[:, 0:64],
                                        rh[:, sub * 128 : (sub + 1) * 128],
                                        identb[0:64, 0:64],
                                    )
                                    nc.vector.tensor_copy(xs4[:, sub, :], tpp[:, 0:64])
                                kb0 = nf0 // 128
                                o0 = (b * NB + kb0) * UNITS
                                nc.gpsimd.dma_start(
                                    x0p[:, o0 : o0 + 4 * UNITS],
                                    xs4.rearrange("p s u -> p (s u)"),
                                )
                            else:
                                ct = aux.tile([64, 512], F32, name="ct", tag="ct")
                                nc.scalar.activation(
                                    ct, zp, mybir.ActivationFunctionType.Tanh, bias=bg_sb
                                )
                                tmp = aux.tile([64, 512], F32, name="tmp", tag="tmp")
                                nc.vector.tensor_sub(
                                    tmp, hx_t[:, nfc * 512 : (nfc + 1) * 512], ct
                                )
                                nc.vector.tensor_mul(
                                    tmp, tmp, u_t[:, nfc * 512 : (nfc + 1) * 512]
                                )
                                ot = aux.tile([64, 512], F32, name="ot", tag="ot")
                                nc.vector.tensor_add(ot, tmp, ct)
                                nc.gpsimd.dma_start(outt[b, :, nf0 : nf0 + 512], ot)

        diffusion(0)
        projection(0)
        diffusion(1)
        projection(1)

    nc.compile()
    return nc


def _fold_weights(w, out_dim):
    """w: (330, out). Returns [66, 5*out] bf16 with the reference's
    x0c-mutation linear combinations and the fp8 chain storage scale
    (1/Y_SCALE on diffused blocks) folded in, rows reordered hx-first."""
    Wm = w.reshape(66, 5, out_dim)
    ys = 1.0 / Y_SCALE
    What = np.stack(
        [
            Wm[:, 0] - Wm[:, 2],
            (Wm[:, 1] - Wm[:, 4]) * ys,
            2.0 * ys * Wm[:, 2],
            ys * Wm[:, 3],
            2.0 * ys * Wm[:, 4],
        ]
    )  # [5, 66, out]
    What = np.concatenate([What[:, 2:, :], What[:, :2, :]], axis=1)  # hx rows first
    return np.ascontiguousarray(
        What.transpose(1, 0, 2).reshape(66, 5 * out_dim)
    ).astype(NP_BF16)


_NC_CACHE = {}


def _get_nc(N):
    if N not in _NC_CACHE:
        _NC_CACHE[N] = _build_nc(N)
    return _NC_CACHE[N]


def kernel(inputs, hx, supports, w_fn, b_fn, w_g, b_g):
    inputs = np.ascontiguousarray(np.asarray(inputs), dtype=np.float32)
    hx = np.ascontiguousarray(np.asarray(hx), dtype=np.float32)
    supports = np.ascontiguousarray(np.asarray(supports), dtype=np.float32)
    w_fn = np.asarray(w_fn, dtype=np.float32)
    b_fn = np.asarray(b_fn, dtype=np.float32)
    w_g = np.asarray(w_g, dtype=np.float32)
    b_g = np.asarray(b_g, dtype=np.float32)

    N = supports.shape[1]
    NB = N // 128
    nc = _get_nc(N)

    # ---- replicated tensors ----
    # stb[s, nb, kp, kb*128+m] = supports[s][nb*128+m, kb*128+kp] * S_SCALE
    stb = np.ascontiguousarray(
        (supports * np.float32(S_SCALE))
        .reshape(2, NB, 128, NB, 128)
        .transpose(0, 1, 4, 3, 2)
    ).reshape(2, NB, 128, NB * 128).astype(NP_F8)
    wfn_h = _fold_weights(w_fn, 128)
    wg_h = _fold_weights(w_g, 64)
    bfn_h = b_fn.reshape(128, 1).copy()
    bg_h = b_g.reshape(64, 1).copy()

    in_maps = []
    for c in range(NCORES):
        sl = slice(c * BLOC, (c + 1) * BLOC)
        inp_c = inputs[sl].reshape(BLOC, N, IN_DIM)
        hx_c = hx[sl].reshape(BLOC, N, UNITS)
        # X0 [N, 528]: hx cols b*64+u, input cols 512 + b*2 + j
        x0 = np.concatenate(
            [
                hx_c.transpose(1, 0, 2).reshape(N, CHX),
                inp_c.transpose(1, 0, 2).reshape(N, CIN),
            ],
            axis=1,
        )
        x0pm = np.ascontiguousarray(
            x0.reshape(NB, 128, C).transpose(1, 0, 2)
        ).reshape(128, NB * C).astype(NP_F8)
        xin = x0[:, CHX:]
        xint = np.ascontiguousarray(xin.T).astype(NP_BF16)
        hxt = np.ascontiguousarray(hx_c.transpose(0, 2, 1))
        in_maps.append(
            {
                "x0pm": x0pm,
                "stb": stb,
                "xint": xint,
                "hxt": hxt,
                "hxtb": hxt.astype(NP_BF16),
                "wfn": wfn_h,
                "wg": wg_h,
                "bfn": bfn_h,
                "bg": bg_h,
            }
        )

    kernel.last_in_maps = in_maps
    res = run_bass_kernel_spmd(
        nc,
        in_maps,
        core_ids=list(range(NCORES)),
        trace=bool(int(os.environ.get("DCGRU_TRACE", "0"))),
    )

    out = np.empty((B, N * UNITS), np.float32)
    for c in range(NCORES):
        outt = res.results[c]["outt"]  # [BLOC, UNITS, N]
        out[c * BLOC : (c + 1) * BLOC] = outt.transpose(0, 2, 1).reshape(BLOC, -1)
    kernel.last_results = res
    return out
